# revision 6
# baseline (speedup 1.0000x reference)
"""Trainium2 Bass kernel for DiffusionCoordinateInitializer.

Reference computation:
    coords = einsum("bsd,cd->bsc", latent, W) + b          # [B, S, 3]
    x = noise; for t in reversed(range(T)): x = a*x + (1-a)*coords, a=(t+1)/T
which collapses (affine fixed-point iteration) to
    x = A*noise + (1-A)*(coords + b),  A = prod_{t=1..T} t/T = T!/T^T

Strategy: pure data-parallel, 8 cores, 4096 tokens each.  Memory-bound:
each core streams its 32 MiB latent shard once (DMA roofline ~94 us).
Latent arrives as 2 MB HWDGE loads [128, 4096] fp32 (2 tokens/partition;
token = t0 + 256*L + 2p + t').  Per 512-token super-tile the four
(L, t') sub-tiles split across two compute routes so every engine stays
under the DMA roof:

- PE route (3 sub-tiles): ScalarE casts the load to fp16, TensorE
  transposes 16 128x128 chunks per sub-tile into PSUM (fp16 stationary
  loads get FWL), ScalarE copies full banks to SBUF, then skinny
  accumulating matmuls wt16[128,3]^T @ latT[128,128] -> coords^T in PSUM
  (one full 2KB bank per sub-tile: a start=True matmul clears has_written
  for its whole bank, and sharing banks across accumulation groups - or
  letting PE write a bank DVE is reading - is fatal).  VectorE adds the
  pre-scaled noise, DMA out token-transposed; host unswizzles.

- DVE route (1 sub-tile): scalar_tensor_tensor fused multiply-reduce
  acc[p] = sum_d lat16[p,d] * wb16[c][p,d] into a resident outv tile;
  noise added once at the end; one DMA at the end.

Host folds (1-A) into the weights, A*noise + (1-A)*b into the noise
tensors, and pre-swizzles noise/output layouts to match the on-chip
token order.
"""

import numpy as np
from contextlib import ExitStack

import concourse.bass as bass  # noqa: F401
import concourse.tile as tile
from concourse import bacc, mybir
from concourse.bass_utils import run_bass_kernel_spmd

N_CORES = 8
B, S, D = 4, 8192, 2048
TOK = B * S
TPC = TOK // N_CORES            # 4096
P = 128
SUPER = 512
N_SUPER = TPC // SUPER          # 8
N_CHUNK = D // P                # 16
F32 = mybir.dt.float32
F16 = mybir.dt.float16

SUBTILES = [(0, 0), (0, 1), (1, 0), (1, 1)]
N_PE = 3                        # sub-tiles routed via TensorE (rest: DVE)
CAST_ENG = "split"              # one cast on ScalarE, one on VectorE

# tuning knobs (env-overridable for bench harnesses; defaults are shipped)
import os as _os
N_PE = int(_os.environ.get("K_NPE", N_PE))
CAST_ENG = _os.environ.get("K_CAST", CAST_ENG)
BUFS_LAT32 = int(_os.environ.get("K_B32", "4"))
BUFS_LAT16 = int(_os.environ.get("K_B16", "3"))
BUFS_PS = int(_os.environ.get("K_BPS", "4"))
BUFS_CPS = int(_os.environ.get("K_BCPS", "4"))

_NC_CACHE = {}


def pe_subtiles(n_pe):
    order = [(0, 0), (1, 0), (0, 1), (1, 1)]
    return order[:n_pe]


def _build(n_pe=N_PE, cast_eng=CAST_ENG, repeat=1, hw_loop=False):
    key = ("v7", n_pe, cast_eng, repeat, hw_loop,
           BUFS_LAT32, BUFS_LAT16, BUFS_PS, BUFS_CPS)
    if key in _NC_CACHE:
        return _NC_CACHE[key]

    n_d = 4 - n_pe
    pe_subs = pe_subtiles(n_pe)
    d_subs = [st for st in SUBTILES if st not in pe_subs]

    nc = bacc.Bacc("TRN2", target_bir_lowering=False, debug=False,
                   enable_asserts=False, num_devices=N_CORES)
    # latent viewed as [TPC/2, 2*D]: row r holds tokens 2r, 2r+1
    lat = nc.dram_tensor("lat", [TPC // 2, 2 * D], F32, kind="ExternalInput").ap()
    wt = nc.dram_tensor("wt", [P, 3 * N_CHUNK], F16, kind="ExternalInput").ap()
    ident = nc.dram_tensor("ident", [P, P], F16, kind="ExternalInput").ap()
    if n_pe:
        nzt = nc.dram_tensor("nzt", [3, n_pe * P * N_SUPER], F32,
                             kind="ExternalInput").ap()
        outpe = nc.dram_tensor("outpe", [3, n_pe * P * N_SUPER], F32,
                               kind="ExternalOutput").ap()
    if n_d:
        wb = nc.dram_tensor("wb", [P, 3 * D], F16, kind="ExternalInput").ap()
        nzv = nc.dram_tensor("nzv", [P, N_SUPER * n_d * 3], F32,
                             kind="ExternalInput").ap()
        outv = nc.dram_tensor("outv", [P, N_SUPER * n_d * 3], F32,
                              kind="ExternalOutput").ap()

    with tile.TileContext(nc) as tc:
        with ExitStack() as ctx:
            const = ctx.enter_context(tc.tile_pool(name="const", bufs=1))
            lat32_pool = ctx.enter_context(tc.tile_pool(name="lat32", bufs=BUFS_LAT32))
            lat16_pool = ctx.enter_context(tc.tile_pool(name="lat16", bufs=BUFS_LAT16))
            latT_pool = ctx.enter_context(tc.tile_pool(name="latT", bufs=2))
            scr_pool = ctx.enter_context(tc.tile_pool(name="scr", bufs=2))
            ps_pool = ctx.enter_context(tc.tile_pool(name="ps", bufs=BUFS_PS, space="PSUM"))
            cps_pool = ctx.enter_context(tc.tile_pool(name="cps", bufs=BUFS_CPS, space="PSUM"))
            osb_pool = ctx.enter_context(tc.tile_pool(name="osb", bufs=3))

            id_t = const.tile([P, P], F16)
            nc.sync.dma_start(id_t[:], ident[:])
            wt_t = const.tile([P, 3 * N_CHUNK], F16)
            nc.sync.dma_start(wt_t[:], wt[:])
            if n_pe:
                nzt_t = const.tile([3, n_pe * P * N_SUPER], F32)
                nc.sync.dma_start(nzt_t[:], nzt[:])
            if n_d:
                wb_t = const.tile([P, 3 * D], F16)
                nc.sync.dma_start(wb_t[:], wb[:])
                nzv_t = const.tile([P, N_SUPER * n_d * 3], F32)
                nc.sync.dma_start(nzv_t[:], nzv[:])
                outv_t = const.tile([P, N_SUPER * n_d * 3], F32)

            def super_tile(sup):
                lat32 = {}
                for L in range(2):
                    lt = lat32_pool.tile([P, 2 * D], F32, name="lt32", tag="lt32")
                    r0 = sup * (SUPER // 2) + L * P
                    nc.sync.dma_start(lt[:], lat[r0:r0 + P, :])
                    lat32[L] = lt

                lat16 = {}
                for L in range(2):
                    lt16 = lat16_pool.tile([P, 2 * D], F16, name="lt16", tag="lt16")
                    if cast_eng == "scalar":
                        nc.scalar.copy(lt16[:], lat32[L][:])
                    elif cast_eng == "vector":
                        nc.vector.tensor_copy(lt16[:], lat32[L][:])
                    else:  # split
                        if L == 0:
                            nc.scalar.copy(lt16[:], lat32[L][:])
                        else:
                            nc.vector.tensor_copy(lt16[:], lat32[L][:])
                    lat16[L] = lt16

                if n_pe:
                    latT = latT_pool.tile([P, n_pe * D], F16,
                                          name="latT", tag="latT")
                    for j, (L, tp) in enumerate(pe_subs):
                        src16 = lat16[L]
                        for g in range(2):
                            ps = ps_pool.tile([P, 8 * P], F16, name="ps", tag="ps")
                            for dk in range(8):
                                k = g * 8 + dk
                                nc.tensor.transpose(
                                    ps[:, dk * P:(dk + 1) * P],
                                    src16[:, tp * D + k * P:tp * D + (k + 1) * P],
                                    id_t[:])
                            nc.scalar.copy(
                                latT[:, j * D + g * 8 * P:j * D + (g + 1) * 8 * P],
                                ps[:])

                    osb = osb_pool.tile([3, n_pe * P], F32, name="osb", tag="osb")
                    c0 = sup * n_pe * P
                    for j in range(n_pe):
                        cps = cps_pool.tile([3, 512], F32, name="cps", tag="cps")
                        for k in range(N_CHUNK):
                            nc.tensor.matmul(
                                cps[:, :P],
                                wt_t[:, k * 3:(k + 1) * 3],
                                latT[:, j * D + k * P:j * D + (k + 1) * P],
                                start=(k == 0), stop=(k == N_CHUNK - 1),
                            )
                        nc.vector.tensor_add(
                            osb[:, j * P:(j + 1) * P], cps[:, :P],
                            nzt_t[:, c0 + j * P:c0 + (j + 1) * P])
                    nc.sync.dma_start(outpe[:, c0:c0 + n_pe * P], osb[:])

                for jd, (L, tp) in enumerate(d_subs):
                    col0 = (sup * n_d + jd) * 3
                    src = lat16[L][:, tp * D:(tp + 1) * D]
                    scr = scr_pool.tile([P, D], F16, name="scr", tag="scr")
                    for c in range(3):
                        nc.vector.scalar_tensor_tensor(
                            out=scr[:],
                            in0=src,
                            scalar=1.0,
                            in1=wb_t[:, c * D:(c + 1) * D],
                            op0=mybir.AluOpType.mult,
                            op1=mybir.AluOpType.mult,
                            accum_out=outv_t[:, col0 + c:col0 + c + 1],
                        )

            if hw_loop and repeat > 1:
                with tc.For_i(0, repeat, 1):
                    for sup in range(N_SUPER):
                        super_tile(sup)
            else:
                for sup_r in range(N_SUPER * repeat):
                    super_tile(sup_r % N_SUPER)

            if n_d:
                nc.vector.tensor_add(outv_t[:], outv_t[:], nzv_t[:])
                nc.sync.dma_start(outv[:], outv_t[:])

    nc.compile()
    _NC_CACHE[key] = nc
    return nc


def _coeff(T):
    a = 1.0
    for t in range(T):
        a *= (t + 1) / T
    return a


def _token_maps(n_pe):
    pe_subs = pe_subtiles(n_pe)
    d_subs = [st for st in SUBTILES if st not in pe_subs]
    sup = np.arange(N_SUPER)[:, None, None]
    p = np.arange(P)[None, None, :]

    def toks(subs):
        if not subs:
            return np.zeros((N_SUPER, 0, P), np.int64)
        Ls = np.array([L for (L, _) in subs])[None, :, None]
        tps = np.array([tp for (_, tp) in subs])[None, :, None]
        return sup * SUPER + Ls * 256 + 2 * p + tps

    return toks(pe_subs), toks(d_subs)


def prep_inputs(latent, W, b, noise, diffusion_steps, n_pe=N_PE):
    T = int(diffusion_steps)
    A = _coeff(T)
    n_d = 4 - n_pe
    lat_flat = np.ascontiguousarray(latent.reshape(TOK, D), dtype=np.float32)
    w_eff = (np.float32(1.0 - A) * W.astype(np.float32))       # [3, D]
    nz_eff = (np.float32(A) * noise.reshape(TOK, 3).astype(np.float32)
              + np.float32(1.0 - A) * b.astype(np.float32)[None, :])

    wt_eff_T = np.ascontiguousarray(w_eff.T)                   # [D, 3]
    wt_packed = np.ascontiguousarray(
        wt_eff_T.reshape(N_CHUNK, P, 3).transpose(1, 0, 2).reshape(P, 3 * N_CHUNK)
    ).astype(np.float16)
    ident = np.eye(P, dtype=np.float16)
    wb = np.ascontiguousarray(
        np.broadcast_to(w_eff.reshape(1, 3 * D), (P, 3 * D))).astype(np.float16)

    tok_pe, tok_d = _token_maps(n_pe)
    in_maps = []
    for c in range(N_CORES):
        shard_nz = nz_eff[c * TPC:(c + 1) * TPC]
        im = {
            "lat": lat_flat[c * TPC:(c + 1) * TPC].reshape(TPC // 2, 2 * D),
            "wt": wt_packed,
            "ident": ident,
        }
        if n_pe:
            im["nzt"] = np.ascontiguousarray(shard_nz[tok_pe.reshape(-1)].T)
        if n_d:
            im["wb"] = wb
            nzv = shard_nz[tok_d]                              # [S, n_d, P, 3]
            im["nzv"] = np.ascontiguousarray(
                nzv.transpose(2, 0, 1, 3).reshape(P, N_SUPER * n_d * 3))
        in_maps.append(im)
    return in_maps, (tok_pe, tok_d)


def assemble_output(results, tok_maps, n_pe=N_PE):
    tok_pe, tok_d = tok_maps
    n_d = 4 - n_pe
    out = np.empty((TOK, 3), dtype=np.float32)
    for c in range(N_CORES):
        oc = out[c * TPC:(c + 1) * TPC]
        if n_pe:
            ope = results[c]["outpe"]
            oc[tok_pe.reshape(-1)] = ope.T
        if n_d:
            ov = results[c]["outv"]
            ov = ov.reshape(P, N_SUPER, n_d, 3).transpose(1, 2, 0, 3)
            oc[tok_d] = ov
    return out.reshape(B, S, 3)


def kernel(latent, W, b, noise, diffusion_steps, _trace=False):
    nc = _build()
    in_maps, tok_maps = prep_inputs(latent, W, b, noise, diffusion_steps)
    res = run_bass_kernel_spmd(nc, in_maps, core_ids=list(range(N_CORES)),
                               trace=_trace)
    if _trace:
        kernel._last_results = res
    return assemble_output(res.results, tok_maps)


# hooks for bench2.py-style timing harnesses
def build(repeat=1, hw_loop=False):
    return _build(repeat=repeat, hw_loop=hw_loop)


def bench_in_maps(rng):
    latent = rng.standard_normal((B, S, D), dtype=np.float32)
    W = (rng.standard_normal((3, D)) * (D ** -0.5)).astype(np.float32)
    b = np.zeros(3, np.float32)
    noise = rng.standard_normal((B, S, 3), dtype=np.float32)
    in_maps, _ = prep_inputs(latent, W, b, noise, 10)
    return in_maps
